# revision 14
# baseline (speedup 1.0000x reference)
# Trainium2 Bass kernel for nn_HamEvo_56006373540016.
#
# Math: the reference integrates ds/dt = -i H s with RK4 (10 steps, 4 stages)
# where H acts only on qubits (18, 19) of a 20-qubit state — i.e. a 4x4
# complex matrix per batch element applied along the "s" axis of
# state[x, s, b] (x = 2^18 spectator index, s = 4, b = 16 batch).
# RK4 on a LINEAR ODE is exactly the degree-4 Taylor polynomial of exp(hA),
# so the whole 10-step evolution collapses to one 4x4 complex matrix per
# batch: E_b = (I + hA + (hA)^2/2 + (hA)^3/6 + (hA)^4/24)^10, A = -i G_b.
# We precompute E_b on the host in float64, realify it into an 8x8 real block
# (acting on [re(4); im(4)]), and assemble a 128x128 block-diagonal weight
# over the 16 batches. The device kernel is then a single streamed matmul:
#   Y[128, x] = W[128, 128] @ X[128, x]      (partition dim = (b, c, s))
# which reads the state once and writes it once — memory-bound.
#
# fp32 runs at ~345 GB/s/core = 96% of the ~358 GB/s HBM-per-NeuronCore
# limit, so the only real lever is fewer bytes: state and weight are cast
# to float16 on the host (rel-err ~4e-4 vs the 2e-2 gate), halving both
# streams. PSUM accumulation stays fp32.
#
# Pipeline shape (from NTFF traces): the launch barrier is the engines'
# instruction fetch (small program = early start); the weight rides the
# first x tile (a separate [128,128] DMA is descriptor-bound and delays
# the first matmul ~2.5us); the kernel end is paced by the OUT stream,
# which runs ~15us behind the in stream — so x tiles ramp up quickly to
# 2 MiB, while y stores start small (to prime the out queue early) and
# END with dense 2 MiB stores split across both HWDGE rings so per-DMA
# completion stalls on one ring hide under data on the other.
#
# Sharding: the x axis (2^18 values) is split contiguously across 8 cores
# (zero communication; every core gets all batches and the same weight).

import numpy as np

P = 128
B = 16
S = 4
X18 = 1 << 18            # number of x values (qubits 0..17)
NCORES = 8
XC = X18 // NCORES       # 32768 x values per core
FT = 8192                # free elems per bulk tile ([128, FT] f16 = 2 MiB)
MM = 512                 # matmul free dim (one PSUM bank of fp32)
PB = 2048                # psum group: 4 banks of 512 fp32

X_SIZES = [1024, 1024, 2048, 4096, 8192, 8192, 8192]
Y_SIZES = [512, 1536, 2048, 4096, 8192, 8192, 8192]

_PERM = np.array([0, 2, 1, 3])  # bit-swap of the 2-qubit index (pyqtorch order)

_NC_CACHE = {}


def _build_nc():
    """Build the Bass program (same SPMD program for all 8 cores)."""
    import concourse.mybir as mybir
    from concourse import bacc
    from concourse.tile import TileContext

    assert sum(X_SIZES) == XC and sum(Y_SIZES) == XC
    nc = bacc.Bacc(
        "TRN2", target_bir_lowering=False, debug=False, num_devices=NCORES
    )
    f16 = mybir.dt.float16
    # x = [ W (128 cols) | state (XC cols) ]: the weight rides the first
    # (small, fast) state tile instead of its own descriptor-bound DMA.
    x = nc.dram_tensor("x", [P, 128 + XC], f16, kind="ExternalInput")
    y = nc.dram_tensor("y", [P, XC], f16, kind="ExternalOutput")

    with TileContext(nc) as tc:
        with (
            tc.tile_pool(name="wx", bufs=1) as wxp,
            tc.tile_pool(name="warm", bufs=1) as warm,
            tc.tile_pool(name="xin", bufs=6) as xin,
            tc.tile_pool(name="yout", bufs=4) as yout,
            tc.tile_pool(name="ps", bufs=2, space="PSUM") as ps,
        ):
            # Head tile: weight + first X_SIZES[0] state columns, one DMA.
            # Own pool so the weight slice stays resident all kernel.
            wxt = wxp.tile([P, 128 + X_SIZES[0]], f16)
            nc.sync.dma_start(wxt[:], x[:, :128 + X_SIZES[0]])
            wt = wxt[:, 0:128]

            # PE warm-up: the HAM clock gate keeps an idle PE at 1.2 GHz,
            # and at 1.2 GHz the matmul stream (362 ns / 512-col chunk)
            # paces y production below what the out-queue can drain.
            # ~10 dummy matmuls on scratch SBUF keep the PE busy through
            # the ramp window while the first x tile is still in flight,
            # so the real matmuls run at 2.4 GHz.
            wsrc = warm.tile([P, MM], f16)
            nc.vector.memset(wsrc[:], 0.0)
            wps = ps.tile([P, PB], mybir.dt.float32, tag="pt")
            for _ in range(10):
                nc.tensor.matmul(wps[:, 0:MM], wsrc[:, 0:128], wsrc)

            xi = yi = 0          # current x / y tile index
            xt, xoff = wxt, 128  # SBUF tile holding current x columns
            xleft = X_SIZES[0]
            yt = yout.tile([P, FT], f16, tag="yt")
            ybase, yleft, ycols = 0, Y_SIZES[0], Y_SIZES[0]
            ncopy = 0
            for g in range(0, XC, PB):
                pb = min(PB, XC - g)
                pt = ps.tile([P, PB], mybir.dt.float32, tag="pt")
                for j in range(0, pb, MM):
                    if xleft == 0:
                        xi += 1
                        ft = X_SIZES[xi]
                        xt = xin.tile([P, FT], f16, tag="xt")
                        a = 128 + g + j
                        nc.sync.dma_start(xt[:, :ft], x[:, a:a + ft])
                        xoff, xleft = -(g + j), ft
                    # One K=128 matmul per 512-col chunk: the block-diag
                    # zeros are free (PE time = streamed columns), and it
                    # quarters the instruction count vs quadrant tiling.
                    nc.tensor.matmul(
                        pt[:, j:j + MM],
                        wt,
                        xt[:, xoff + g + j:xoff + g + j + MM],
                    )
                    xleft -= MM
                # PSUM->SBUF (fp32->fp16) copies, segmented at y-tile
                # boundaries and alternated between the DVE and ACT
                # engines so neither paces the stream.
                seg = g
                while seg < g + pb:
                    se = min(g + pb, ybase + ycols)
                    yo = seg - ybase
                    if ncopy % 2 == 0:
                        nc.vector.tensor_copy(
                            yt[:, yo:yo + se - seg], pt[:, seg - g:se - g]
                        )
                    else:
                        nc.scalar.copy(
                            yt[:, yo:yo + se - seg], pt[:, seg - g:se - g]
                        )
                    ncopy += 1
                    yleft -= se - seg
                    seg = se
                    if yleft > 0:
                        continue
                    # y tile complete -> store on the ACT ring (Sync stays
                    # dedicated to the in-DMAs mid-kernel: a y-store's sem
                    # wait there would block them). The LAST tile completes
                    # after every in-trigger has issued, so splitting it
                    # across both HWDGE rings is safe and halves the drain.
                    if yi == len(Y_SIZES) - 1:
                        h = ycols // 2
                        nc.scalar.dma_start(
                            y[:, ybase:ybase + h], yt[:, :h]
                        )
                        nc.sync.dma_start(
                            y[:, ybase + h:ybase + ycols], yt[:, h:ycols]
                        )
                    else:
                        nc.scalar.dma_start(
                            y[:, ybase:ybase + ycols], yt[:, :ycols]
                        )
                    ybase += ycols
                    yi += 1
                    if yi < len(Y_SIZES):
                        yt = yout.tile([P, FT], f16, tag="yt")
                        ycols = yleft = Y_SIZES[yi]
    nc.compile()
    return nc


def _get_nc():
    if "nc" not in _NC_CACHE:
        _NC_CACHE["nc"] = _build_nc()
    return _NC_CACHE["nc"]


def _build_weight(H_re, H_im, t):
    """128x128 block-diag weight: per-batch realified 10-step RK4 evolution."""
    H = H_re.astype(np.float64) + 1j * H_im.astype(np.float64)  # (4,4,B)
    G = H[_PERM][:, _PERM]  # memory-order gate: G[s_out, s_in, b]
    # reference computes h = t / 10 in float32
    h = (t.astype(np.float32) / np.float32(10)).astype(np.float64)
    I4 = np.eye(S, dtype=np.complex128)
    W = np.zeros((P, P), np.float64)
    for b in range(B):
        M = (-1j) * h[b] * G[:, :, b]
        R = I4 + M + M @ M / 2 + M @ M @ M / 6 + M @ M @ M @ M / 24
        E = np.linalg.matrix_power(R, 10)
        W[b * 8:(b + 1) * 8, b * 8:(b + 1) * 8] = np.block(
            [[E.real, -E.imag], [E.imag, E.real]]
        )
    return W


LAST_RESULT = None


def _run(inputs, trace=False, trace_cores=None, tmpdir=None):
    global LAST_RESULT
    from concourse.bass_utils import run_bass_kernel_spmd

    W = _build_weight(inputs["H_re"], inputs["H_im"], inputs["t"])
    lhsT = W.T.astype(np.float16)  # matmul computes lhsT.T @ rhs

    # Repack state into [p, x] with p = b*8 + c*4 + s, cast to fp16.
    sr = np.asarray(inputs["state_re"], np.float32).reshape(X18, S, B)
    si = np.asarray(inputs["state_im"], np.float32).reshape(X18, S, B)
    A = np.empty((B, 2, S, X18), np.float16)
    A[:, 0] = sr.transpose(2, 1, 0)
    A[:, 1] = si.transpose(2, 1, 0)
    A = A.reshape(P, X18)

    in_maps = []
    for c in range(NCORES):
        xw = np.empty((P, 128 + XC), np.float16)
        xw[:, :128] = lhsT
        xw[:, 128:] = A[:, c * XC:(c + 1) * XC]
        in_maps.append({"x": xw})

    nc = _get_nc()
    res = run_bass_kernel_spmd(
        nc,
        in_maps,
        list(range(NCORES)),
        trace=trace,
        trace_cores=trace_cores,
        tmpdir=tmpdir,
    )
    LAST_RESULT = res

    Y = np.empty((P, X18), np.float32)
    for c in range(NCORES):
        Y[:, c * XC:(c + 1) * XC] = res.results[c]["y"]

    y4 = Y.reshape(B, 2, S, X18)
    out_shape = (2,) * 20 + (B,)
    out = np.empty((2,) + out_shape, np.float32)
    out[0] = y4[:, 0].transpose(2, 1, 0).reshape(out_shape)
    out[1] = y4[:, 1].transpose(2, 1, 0).reshape(out_shape)
    return out, res.exec_time_ns


def kernel(**inputs):
    out, _ = _run(inputs, trace=False)
    return out


# revision 15
# speedup vs baseline: 1.1385x; 1.1385x over previous
# Trainium2 Bass kernel for nn_HamEvo_56006373540016.
#
# Math: the reference integrates ds/dt = -i H s with RK4 (10 steps, 4 stages)
# where H acts only on qubits (18, 19) of a 20-qubit state — i.e. a 4x4
# complex matrix per batch element applied along the "s" axis of
# state[x, s, b] (x = 2^18 spectator index, s = 4, b = 16 batch).
# RK4 on a LINEAR ODE is exactly the degree-4 Taylor polynomial of exp(hA),
# so the whole 10-step evolution collapses to one 4x4 complex matrix per
# batch: E_b = (I + hA + (hA)^2/2 + (hA)^3/6 + (hA)^4/24)^10, A = -i G_b.
# We precompute E_b on the host in float64, realify it into an 8x8 real block
# (acting on [re(4); im(4)]), and assemble a 128x128 block-diagonal weight
# over the 16 batches. The device kernel is then a single streamed matmul:
#   Y[128, x] = W[128, 128] @ X[128, x]      (partition dim = (b, c, s))
# which reads the state once and writes it once — memory-bound.
#
# fp32 runs at ~345 GB/s/core = 96% of the ~358 GB/s HBM-per-NeuronCore
# limit, so the only real lever is fewer bytes: state and weight are cast
# to float16 on the host (rel-err ~4e-4 vs the 2e-2 gate), halving both
# streams. PSUM accumulation stays fp32.
#
# Pipeline shape (from NTFF traces): the launch barrier is the engines'
# instruction fetch (small program = early start); the weight rides the
# first x tile (a separate [128,128] DMA is descriptor-bound and delays
# the first matmul ~2.5us); the kernel end is paced by the OUT stream,
# which runs ~15us behind the in stream — so x tiles ramp up quickly to
# 2 MiB, while y stores start small (to prime the out queue early) and
# END with dense 2 MiB stores split across both HWDGE rings so per-DMA
# completion stalls on one ring hide under data on the other.
#
# Sharding: the x axis (2^18 values) is split contiguously across 8 cores
# (zero communication; every core gets all batches and the same weight).

import numpy as np

P = 128
B = 16
S = 4
X18 = 1 << 18            # number of x values (qubits 0..17)
NCORES = 8
XC = X18 // NCORES       # 32768 x values per core
FT = 8192                # free elems per bulk tile ([128, FT] f16 = 2 MiB)
MM = 512                 # matmul free dim (one PSUM bank of fp32)
PB = 2048                # psum group: 4 banks of 512 fp32

X_SIZES = [1024, 1024, 2048, 4096, 8192, 8192, 8192]
Y_SIZES = [512, 1536, 2048, 4096, 8192, 8192, 8192]

_PERM = np.array([0, 2, 1, 3])  # bit-swap of the 2-qubit index (pyqtorch order)

_NC_CACHE = {}


def _build_nc():
    """Build the Bass program (same SPMD program for all 8 cores)."""
    import concourse.mybir as mybir
    from concourse import bacc
    from concourse.tile import TileContext

    assert sum(X_SIZES) == XC and sum(Y_SIZES) == XC
    nc = bacc.Bacc(
        "TRN2", target_bir_lowering=False, debug=False, num_devices=NCORES
    )
    f16 = mybir.dt.float16
    # x = [ W (128 cols) | state (XC cols) ]: the weight rides the first
    # (small, fast) state tile instead of its own descriptor-bound DMA.
    x = nc.dram_tensor("x", [P, 128 + XC], f16, kind="ExternalInput")
    y = nc.dram_tensor("y", [P, XC], f16, kind="ExternalOutput")

    with TileContext(nc) as tc:
        with (
            tc.tile_pool(name="wx", bufs=1) as wxp,
            tc.tile_pool(name="xin", bufs=5) as xin,
            tc.tile_pool(name="yout", bufs=6) as yout,
            tc.tile_pool(name="ps", bufs=2, space="PSUM") as ps,
        ):
            # Head tile: weight + first X_SIZES[0] state columns, one DMA.
            # Own pool so the weight slice stays resident all kernel.
            # (yout is deep — 6 of 7 y tiles can sit completed in SBUF —
            # because the HAM clock gate throttles the idle-ish PE to
            # 1.2 GHz, so y production is bursty; buffering keeps the
            # out-queue from running dry in the drain phase.)
            wxt = wxp.tile([P, 128 + X_SIZES[0]], f16)
            nc.sync.dma_start(wxt[:], x[:, :128 + X_SIZES[0]])
            wt = wxt[:, 0:128]

            xi = yi = 0          # current x / y tile index
            xt, xoff = wxt, 128  # SBUF tile holding current x columns
            xleft = X_SIZES[0]
            yt = yout.tile([P, FT], f16, tag="yt")
            ybase, yleft, ycols = 0, Y_SIZES[0], Y_SIZES[0]
            ncopy = 0
            for g in range(0, XC, PB):
                pb = min(PB, XC - g)
                pt = ps.tile([P, PB], mybir.dt.float32, tag="pt")
                for j in range(0, pb, MM):
                    if xleft == 0:
                        xi += 1
                        ft = X_SIZES[xi]
                        xt = xin.tile([P, FT], f16, tag="xt")
                        a = 128 + g + j
                        nc.sync.dma_start(xt[:, :ft], x[:, a:a + ft])
                        xoff, xleft = -(g + j), ft
                    # One K=128 matmul per 512-col chunk: the block-diag
                    # zeros are free (PE time = streamed columns), and it
                    # quarters the instruction count vs quadrant tiling.
                    nc.tensor.matmul(
                        pt[:, j:j + MM],
                        wt,
                        xt[:, xoff + g + j:xoff + g + j + MM],
                    )
                    xleft -= MM
                # PSUM->SBUF (fp32->fp16) copies, segmented at y-tile
                # boundaries and alternated between the DVE and ACT
                # engines so neither paces the stream.
                seg = g
                while seg < g + pb:
                    se = min(g + pb, ybase + ycols)
                    yo = seg - ybase
                    if ncopy % 2 == 0:
                        nc.vector.tensor_copy(
                            yt[:, yo:yo + se - seg], pt[:, seg - g:se - g]
                        )
                    else:
                        nc.scalar.copy(
                            yt[:, yo:yo + se - seg], pt[:, seg - g:se - g]
                        )
                    ncopy += 1
                    yleft -= se - seg
                    seg = se
                    if yleft > 0:
                        continue
                    # y tile complete -> store on the ACT ring (Sync stays
                    # dedicated to the in-DMAs mid-kernel: a y-store's sem
                    # wait there would block them). The LAST tile completes
                    # after every in-trigger has issued, so splitting it
                    # across both HWDGE rings is safe and halves the drain.
                    if yi == len(Y_SIZES) - 1:
                        h = ycols // 2
                        nc.scalar.dma_start(
                            y[:, ybase:ybase + h], yt[:, :h]
                        )
                        nc.sync.dma_start(
                            y[:, ybase + h:ybase + ycols], yt[:, h:ycols]
                        )
                    else:
                        nc.scalar.dma_start(
                            y[:, ybase:ybase + ycols], yt[:, :ycols]
                        )
                    ybase += ycols
                    yi += 1
                    if yi < len(Y_SIZES):
                        yt = yout.tile([P, FT], f16, tag="yt")
                        ycols = yleft = Y_SIZES[yi]
    nc.compile()
    return nc


def _get_nc():
    if "nc" not in _NC_CACHE:
        _NC_CACHE["nc"] = _build_nc()
    return _NC_CACHE["nc"]


def _build_weight(H_re, H_im, t):
    """128x128 block-diag weight: per-batch realified 10-step RK4 evolution."""
    H = H_re.astype(np.float64) + 1j * H_im.astype(np.float64)  # (4,4,B)
    G = H[_PERM][:, _PERM]  # memory-order gate: G[s_out, s_in, b]
    # reference computes h = t / 10 in float32
    h = (t.astype(np.float32) / np.float32(10)).astype(np.float64)
    I4 = np.eye(S, dtype=np.complex128)
    W = np.zeros((P, P), np.float64)
    for b in range(B):
        M = (-1j) * h[b] * G[:, :, b]
        R = I4 + M + M @ M / 2 + M @ M @ M / 6 + M @ M @ M @ M / 24
        E = np.linalg.matrix_power(R, 10)
        W[b * 8:(b + 1) * 8, b * 8:(b + 1) * 8] = np.block(
            [[E.real, -E.imag], [E.imag, E.real]]
        )
    return W


LAST_RESULT = None


def _run(inputs, trace=False, trace_cores=None, tmpdir=None):
    global LAST_RESULT
    from concourse.bass_utils import run_bass_kernel_spmd

    W = _build_weight(inputs["H_re"], inputs["H_im"], inputs["t"])
    lhsT = W.T.astype(np.float16)  # matmul computes lhsT.T @ rhs

    # Repack state into [p, x] with p = b*8 + c*4 + s, cast to fp16.
    sr = np.asarray(inputs["state_re"], np.float32).reshape(X18, S, B)
    si = np.asarray(inputs["state_im"], np.float32).reshape(X18, S, B)
    A = np.empty((B, 2, S, X18), np.float16)
    A[:, 0] = sr.transpose(2, 1, 0)
    A[:, 1] = si.transpose(2, 1, 0)
    A = A.reshape(P, X18)

    in_maps = []
    for c in range(NCORES):
        xw = np.empty((P, 128 + XC), np.float16)
        xw[:, :128] = lhsT
        xw[:, 128:] = A[:, c * XC:(c + 1) * XC]
        in_maps.append({"x": xw})

    nc = _get_nc()
    res = run_bass_kernel_spmd(
        nc,
        in_maps,
        list(range(NCORES)),
        trace=trace,
        trace_cores=trace_cores,
        tmpdir=tmpdir,
    )
    LAST_RESULT = res

    Y = np.empty((P, X18), np.float32)
    for c in range(NCORES):
        Y[:, c * XC:(c + 1) * XC] = res.results[c]["y"]

    y4 = Y.reshape(B, 2, S, X18)
    out_shape = (2,) * 20 + (B,)
    out = np.empty((2,) + out_shape, np.float32)
    out[0] = y4[:, 0].transpose(2, 1, 0).reshape(out_shape)
    out[1] = y4[:, 1].transpose(2, 1, 0).reshape(out_shape)
    return out, res.exec_time_ns


def kernel(**inputs):
    out, _ = _run(inputs, trace=False)
    return out
